# revision 19
# baseline (speedup 1.0000x reference)
"""CompositePerturbation Trainium2 kernel.

Pipeline per sample (batch sharded 4-per-core across 8 cores):
  1. Gaussian blur (separable 25-tap, per-sample sigma, gated by flag0) as two
     banded-matmul passes on the tensor engine. The band matrix (or identity
     when blur is off) is host-built as a [128, 896] sliding-window tensor B
     with B[r, t] = f(t - 384 - r), so every 128x128 block / 128x512 row-block
     of the full 512x512 Toeplitz operator is a free-dim slice of B.
  2. Glare: rank-1 gy (x) gx term injected straight into the pass-2 PSUM
     accumulation as a K=1 matmul (intensity and flag folded into gy).
  3. Occlusion: rank-1 (-BIG*ro) (x) co injected into the rain-count PSUM so
     exp() underflows to 0 inside the rectangle -> DO = decay * occl_mask.
  4. Rain: count = rank-60 matmul of host-built row/col streak masks (alpha,
     active-count and flag folded in); decay D = exp(L*count) on ScalarE.
  5. Salt/pepper: mask compares on DVE, min/max application split DVE/GPSIMD.

Elementwise chain per pixel (z = blur + glare, PSUM):
  t = min(z, 1) * DO          (DVE scalar_tensor_tensor, PSUM source)
  u = (t + 1) - D             (GPSIMD stt; == t*O*D + (1-D) given t already *DO)
  w = min(u, M1)              (DVE stt; M1 = (noise >= lo) * 1e9  -> pepper=0)
  o = max(w, M2)              (GPSIMD stt; M2 = (noise > hi)      -> salt=1)
"""

import numpy as np

B, C, H, W = 32, 3, 512, 512
NCORES = 8
BPC = B // NCORES  # samples per core
KS = 25
HALF = KS // 2  # 12
P = 128
NT = H // P  # 4 partition tiles per image
BIGNEG = 100.0  # exp(-100) == 0 in fp32

_CACHE = {}


def _host_params(x, sigma_u, glare_u, occ_u, rain_u, rain_n_u, rain_alpha_u,
                 noise_u, noise_amt_u, apply_flags):
    """Fold every per-sample parameter + apply flag into dense device inputs."""
    f32 = np.float32
    flags = apply_flags.astype(np.int64)

    # ---- blur band tensor B [B, 128, 896]: B[r, t] = f(t - 384 - r) ----
    sigma = 1.0 + 3.0 * sigma_u.astype(np.float64)
    coords = np.arange(KS, dtype=np.float64) - HALF
    g = np.exp(-coords[None, :] ** 2 / (2.0 * sigma[:, None] ** 2))
    g = (g / g.sum(axis=1, keepdims=True)).astype(f32)  # [B, 25]
    band = np.zeros((B, P, 896), dtype=f32)
    r = np.arange(P)[:, None]
    t = np.arange(896)[None, :]
    d = t - 384 - r  # displacement h' - h
    inband = np.abs(d) <= HALF
    for b in range(B):
        if flags[b, 0] > 0:
            vals = np.zeros((P, 896), dtype=f32)
            vals[inband] = g[b][(d[inband] + HALF)]
            band[b] = vals
        else:
            band[b] = (d == 0).astype(f32)

    # ---- glare vectors gy (intensity folded), gx ----
    yy = np.arange(H, dtype=f32)
    xx = np.arange(W, dtype=f32)
    inten = 0.4 + 0.5 * glare_u[:, 0]
    rx = (0.1 + 0.25 * glare_u[:, 1]) * W / 2
    ry = (0.1 + 0.25 * glare_u[:, 2]) * H / 2
    cx = (0.2 + 0.6 * glare_u[:, 3]) * W
    cy = (0.2 + 0.6 * glare_u[:, 4]) * H
    gy = np.exp(-((yy[None, :] - cy[:, None]) / ry[:, None]) ** 2)
    gx = np.exp(-((xx[None, :] - cx[:, None]) / rx[:, None]) ** 2)
    gy = (gy * inten[:, None] * (flags[:, 1] > 0)[:, None]).astype(f32)  # [B, 512]
    gx = gx.astype(f32)

    # ---- occlusion row/col indicators ----
    ph = np.floor(H * (0.1 + 0.3 * occ_u[:, 0]))
    pw = np.floor(W * (0.1 + 0.3 * occ_u[:, 1]))
    y0 = np.floor(occ_u[:, 2] * (H - ph))
    x0 = np.floor(occ_u[:, 3] * (W - pw))
    ro = ((yy[None, :] >= y0[:, None]) & (yy[None, :] < (y0 + ph)[:, None]))
    co = ((xx[None, :] >= x0[:, None]) & (xx[None, :] < (x0 + pw)[:, None]))
    ro = (ro & (flags[:, 2] > 0)[:, None]).astype(f32)
    co = co.astype(f32)
    robig = (-BIGNEG) * ro  # [B, 512]

    # ---- rain streak masks, alpha/L folded ----
    S = rain_u.shape[1]
    n = np.floor(20.0 + 41.0 * rain_n_u)
    a = 0.15 + 0.35 * rain_alpha_u
    L = np.where(flags[:, 3] > 0, np.log(1.0 - a), 0.0)
    xc = np.floor(rain_u[:, :, 0] * W)
    y0s = np.floor(rain_u[:, :, 1] * (H // 2))
    y1s = (H // 2) + np.floor(rain_u[:, :, 2] * (H // 2))
    hh = np.arange(H, dtype=f32)[None, None, :]
    ww = np.arange(W, dtype=f32)[None, None, :]
    rowm = ((hh >= y0s[:, :, None]) & (hh < y1s[:, :, None])).astype(f32)
    colm = ((ww >= xc[:, :, None] - 1) & (ww <= xc[:, :, None])).astype(f32)
    active = (np.arange(S)[None, :] < n[:, None]).astype(f32)
    rowm_aL = rowm * active[:, :, None] * L[:, None, None].astype(f32)
    rain = np.concatenate([rowm_aL, colm], axis=2).astype(f32)  # [B, 60, 1024]

    # gvec [B, 2, 1024]: row0 = gy | gx, row1 = robig | co  (rank-2 matmul:
    # psZ += gy (x) gx + robig (x) co injects glare + occlusion in one shot)
    gvec = np.stack([
        np.concatenate([gy, gx], axis=1),
        np.concatenate([robig, co], axis=1),
    ], axis=1).astype(f32)

    # ---- salt/pepper thresholds ----
    amount = 0.01 + 0.07 * noise_amt_u
    lo = np.where(flags[:, 4] > 0, amount / 2, 0.0).astype(f32)
    hi = np.where(flags[:, 4] > 0, 1.0 - amount / 2, 2.0).astype(f32)
    scal = np.zeros((B, P, 3), dtype=f32)
    scal[:, :, 0] = lo[:, None]
    scal[:, :, 1] = hi[:, None]

    ident = np.eye(P, dtype=f32)
    return band, gvec, rain, scal, ident


def _register_dve_ops():
    """Register two fused custom-DVE ops via the documented extension point
    (dve_ops.OPS append; see 04-custom-dve-api.md).

    CPERT_RAIN: out = clamp01(in0) * in1 + 1 - in1
      (in0 = blur+glare-BIG*occl_rect PSUM, in1 = rain decay D;
       the clamp both applies the glare clip and zeroes occluded pixels)
    CPERT_SP:   out = max(select(in0 < s0, 0, in1), in0 > s1)
      (in0 = noise, in1 = rain output; s0 = lo -> pepper, s1 = hi -> salt)
    """
    from concourse import dve_ops
    from concourse.dve_spec import (
        Spec, Src0, Src1, C0, C1, Zero, One, maxx, minn, select, lower,
        _has_src1,
    )
    from concourse.dve_uop import DveOpSpec
    import numpy as np

    if "CPERT_RAIN_ANT" in dve_ops._SUB_OPCODE_FOR_NAME:
        return (dve_ops._BY_NAME_CPERT["CPERT_RAIN_ANT"],
                dve_ops._BY_NAME_CPERT["CPERT_SP_ANT"])

    def make(name, spec):
        row = dve_ops._CUSTOM_DVE_ROW_BASE + len(dve_ops.OPS)
        assert row < 0x20
        shas = {}
        for ver in ("v3", "v4"):
            tmp = DveOpSpec(name=name, opcode=row, uops=lower(spec, ver=ver),
                            rd1_en=_has_src1(spec))
            shas[ver] = tmp.sha(ver)
        op = dve_ops.DveOp(name, spec, False, shas)
        dve_ops._SUB_OPCODE_FOR_NAME[name] = row
        dve_ops.OPS.append(op)
        dve_ops.CUSTOM_DVE_SPECS[name] = spec
        return op

    rain_spec = Spec(
        body=maxx(minn(Src0, One), Zero) * Src1 + One - Src1,
        reference=lambda in0, in1, s0, s1, imm2: (
            np.clip(in0, 0.0, 1.0).astype(np.float32) * in1 + 1.0 - in1
        ).astype(np.float32),
    )
    sp_spec = Spec(
        body=maxx(select(Src0 < C0, Zero, Src1), Src0 > C1),
        reference=lambda in0, in1, s0, s1, imm2: np.maximum(
            np.where(in0 < s0, np.float32(0.0), in1),
            (in0 > s1).astype(np.float32),
        ).astype(np.float32),
    )
    rain_op = make("CPERT_RAIN_ANT", rain_spec)
    sp_op = make("CPERT_SP_ANT", sp_spec)
    dve_ops._BY_NAME_CPERT = {"CPERT_RAIN_ANT": rain_op, "CPERT_SP_ANT": sp_op}
    return rain_op, sp_op


def _build_module():
    import concourse.bacc as bacc
    import concourse.mybir as mybir
    from concourse.tile import TileContext

    f32 = mybir.dt.float32
    f32r = mybir.dt.float32r
    AF = mybir.ActivationFunctionType
    OP = mybir.AluOpType
    # float32r: same 4-byte data, PE streams 1 cycle/row (vs 4 for fp32)
    # when the moving free dim is >= 256. Precision ~bf16 — fine here.
    R = lambda ap: ap.bitcast(f32r)

    RAIN_OP, SP_OP = _register_dve_ops()

    # Bacc (not raw Bass): its compile() pass splits multi-sem waits into
    # event-semaphore chains, satisfying the 1-wait-per-instruction HW limit.
    nc = bacc.Bacc("TRN2", target_bir_lowering=False, debug=False,
                   num_devices=NCORES)
    x_d = nc.declare_dram_parameter("x", [BPC, C, H, W], f32r, isOutput=False)
    n_d = nc.declare_dram_parameter("noise", [BPC, C, H, W], f32, isOutput=False)
    band_d = nc.declare_dram_parameter("band", [BPC, P, 896], f32r, isOutput=False)
    gvec_d = nc.declare_dram_parameter("gvec", [BPC, 2, 1024], f32r, isOutput=False)
    rain_d = nc.declare_dram_parameter("rain", [BPC, 60, 1024], f32r, isOutput=False)
    scal_d = nc.declare_dram_parameter("scal", [BPC, P, 3], f32, isOutput=False)
    id_d = nc.declare_dram_parameter("ident", [P, P], f32r, isOutput=False)
    out_d = nc.declare_dram_parameter("out", [BPC, C, H, W], f32, isOutput=True)

    CW = 3 * W  # 1536, three channels concatenated along free dim

    with TileContext(nc) as tc:
        with (
            tc.tile_pool(name="const", bufs=1) as cpool,
            tc.tile_pool(name="params", bufs=2) as ppool,
            tc.tile_pool(name="xin", bufs=2) as xpool,
            tc.tile_pool(name="ysb", bufs=2) as ypool,
            tc.tile_pool(name="ytsb", bufs=2) as ytpool,
            tc.tile_pool(name="tcat", bufs=5) as tpool,
            tc.tile_pool(name="ncat", bufs=3) as npool,
            tc.tile_pool(name="masks", bufs=2) as mpool,
            tc.tile_pool(name="dd", bufs=5) as dpool,
            tc.tile_pool(name="vps", bufs=2, space="PSUM") as vpsum,
            tc.tile_pool(name="tps", bufs=2, space="PSUM") as tpsum,
            tc.tile_pool(name="zps", bufs=2, space="PSUM") as zpsum,
            tc.tile_pool(name="rps", bufs=1, space="PSUM") as rpsum,
        ):
            ident = cpool.tile([P, P], f32r, tag="ident")
            nc.sync.dma_start(out=ident[:], in_=id_d[:])
            # dummy Exp: absorbs the activation-bias const-tensor dependency
            # (and the ACT table load) so real exps stay under the wait limit
            warm = cpool.tile([P, 2], f32, tag="warm")
            nc.scalar.activation(warm[:, 0:1], ident[:, 0:1].bitcast(f32), AF.Exp)

            for b in range(BPC):
                bandb = ppool.tile([P, 896], f32r, tag="band")
                nc.sync.dma_start(out=bandb[:], in_=band_d[b])
                gv = ppool.tile([2, 1024], f32r, tag="gvec")
                nc.sync.dma_start(out=gv[:], in_=gvec_d[b])
                rn = ppool.tile([60, 1024], f32r, tag="rain")
                nc.sync.dma_start(out=rn[:], in_=rain_d[b])
                sc = ppool.tile([P, 3], f32, tag="scal")
                nc.sync.dma_start(out=sc[:], in_=scal_d[b])
                # DVE-local copy of thresholds so mask ops don't wait on DMA
                sc2 = ppool.tile([P, 3], f32, tag="scal2")
                nc.vector.tensor_copy(sc2[:], sc[:])
                # ACT touch of sc: later exps use sc[:,2:3] as bias without a wait
                wb = ppool.tile([P, 1], f32, tag="warmb")
                nc.scalar.copy(wb[:], sc[:, 2:3])

                # PE touch of param tensors: folds their DMA waits into one
                # tiny matmul each so real matmuls stay under the wait limit.
                junk = rpsum.tile([P, 2], f32, tag="psA", name=f"junk{b}")
                for t_ in (bandb, gv, rn):
                    nc.tensor.matmul(junk[0:1, 0:1], lhsT=t_[0:1, 0:1].bitcast(f32),
                                     rhs=t_[0:1, 0:1].bitcast(f32), start=True, stop=True)

                D_t = []
                # ---- rain decay D = exp(L * count) per h-tile ----
                for u in range(NT):
                    usl = slice(u * P, (u + 1) * P)
                    psA = rpsum.tile([P, W], f32, tag="psA")
                    nc.tensor.matmul(psA[:], lhsT=rn[0:60, usl],
                                     rhs=rn[0:60, 512:1024],
                                     start=True, stop=True)
                    dt_ = dpool.tile([P, W], f32, tag="D")
                    nc.scalar.activation(dt_[:], psA[:], AF.Exp, bias=sc[:, 2:3])
                    D_t.append(dt_)

                tcat = [tpool.tile([P, CW], f32, tag="t", name=f"tcat{b}_{u}")
                        for u in range(NT)]

                for c in range(C):
                    # ---- load x[b, c] as [128, 4*512] (v-tiles along free) ----
                    xt = xpool.tile([P, NT * W], f32r, tag="x")
                    nc.sync.dma_start(
                        out=xt[:].rearrange("p (v w) -> p v w", w=W),
                        in_=x_d[b, c].rearrange("(v p) w -> p v w", p=P),
                    )
                    # ---- pass 1: vertical conv, weights = band blocks ----
                    ysb = ypool.tile([P, NT * W], f32r, tag="y")
                    for v in range(NT):
                        psV = vpsum.tile([P, W], f32, tag="psV")
                        mms = [(bandb[:, 384:512], xt[:, v * W:(v + 1) * W])]
                        if v > 0:
                            mms.append((bandb[:, 512:640], xt[:, (v - 1) * W:v * W]))
                        if v < NT - 1:
                            mms.append((bandb[:, 256:384], xt[:, (v + 1) * W:(v + 2) * W]))
                        for i, (lt, rh) in enumerate(mms):
                            nc.tensor.matmul(psV[:], lhsT=lt, rhs=rh,
                                             start=(i == 0), stop=(i == len(mms) - 1))
                        # ACT copy rounds fp32 PSUM -> f32r SBUF
                        nc.scalar.copy(ysb[:, v * W:(v + 1) * W], psV[:])

                    # ---- transpose Y via PE, evacuate via DMA ----
                    ytsb = ytpool.tile([P, NT * W], f32r, tag="yt")
                    for j in range(NT):
                        psT = tpsum.tile([P, W], f32, tag="psT")
                        for v in range(NT):
                            nc.tensor.transpose(
                                R(psT[:, v * P:(v + 1) * P]),
                                ysb[:, v * W + j * P: v * W + (j + 1) * P],
                                ident[:],
                            )
                        nc.scalar.copy(ytsb[:, j * W:(j + 1) * W], psT[:])

                    # ---- pass 2: horizontal conv + glare + occl, then rain op ----
                    for u in range(NT):
                        usl = slice(u * P, (u + 1) * P)
                        psZ = zpsum.tile([P, W], f32, tag="psZ")
                        for j in range(NT):
                            nc.tensor.matmul(
                                psZ[:],
                                lhsT=ytsb[:, j * W + u * P: j * W + (u + 1) * P],
                                rhs=bandb[:, 384 - j * P: 896 - j * P],
                                start=(j == 0), stop=False,
                            )
                        # glare + occlusion in one rank-2 matmul:
                        # psZ += gy (x) gx + (-BIG*ro) (x) co; the rain op's
                        # clamp01 then zeroes occluded pixels
                        nc.tensor.matmul(psZ[:], lhsT=gv[0:2, usl],
                                         rhs=gv[0:2, 512:1024],
                                         start=False, stop=True)
                        # t = clamp01(z) * D + 1 - D   (fused custom DVE op)
                        nc.vector._custom_dve(
                            RAIN_OP,
                            out=tcat[u][:, c * W:(c + 1) * W],
                            in0=psZ[:], in1=D_t[u][:],
                        )

                # ---- per (b, u): salt/pepper over the 3-channel concat ----
                for u in range(NT):
                    ncat = npool.tile([P, CW], f32, tag="n")
                    nc.sync.dma_start(
                        out=ncat[:].rearrange("p (c w) -> p c w", w=W),
                        in_=n_d[b, :, u * P:(u + 1) * P, :].rearrange("c p w -> p c w"),
                    )
                    ocat = tpool.tile([P, CW], f32, tag="u", name=f"ocat{b}_{u}")
                    nc.vector._custom_dve(
                        SP_OP, out=ocat[:], in0=ncat[:], in1=tcat[u][:],
                        s0=sc2[:, 0:1], s1=sc2[:, 1:2],
                    )
                    nc.sync.dma_start(
                        out=out_d[b, :, u * P:(u + 1) * P, :].rearrange("c p w -> p c w"),
                        in_=ocat[:].rearrange("p (c w) -> p c w", w=W),
                    )
    nc.finalize()
    return nc


def _get_module():
    if "nc" not in _CACHE:
        _CACHE["nc"] = _build_module()
    return _CACHE["nc"]


def kernel(**inputs):
    x = np.asarray(inputs["x"], dtype=np.float32)
    noise = np.asarray(inputs["noise_u"], dtype=np.float32)
    band, gvec, rain, scal, ident = _host_params(
        x, np.asarray(inputs["sigma_u"]), np.asarray(inputs["glare_u"]),
        np.asarray(inputs["occ_u"]), np.asarray(inputs["rain_u"]),
        np.asarray(inputs["rain_n_u"]), np.asarray(inputs["rain_alpha_u"]),
        noise, np.asarray(inputs["noise_amt_u"]),
        np.asarray(inputs["apply_flags"]),
    )

    from concourse.bass_utils import run_bass_kernel_spmd

    nc = _get_module()
    in_maps = []
    for i in range(NCORES):
        s = slice(i * BPC, (i + 1) * BPC)
        in_maps.append({
            "x": np.ascontiguousarray(x[s]),
            "noise": np.ascontiguousarray(noise[s]),
            "band": np.ascontiguousarray(band[s]),
            "gvec": np.ascontiguousarray(gvec[s]),
            "rain": np.ascontiguousarray(rain[s]),
            "scal": np.ascontiguousarray(scal[s]),
            "ident": ident,
        })
    res = run_bass_kernel_spmd(nc, in_maps, list(range(NCORES)))
    _CACHE["last_in_maps"] = in_maps
    out = np.concatenate([r["out"] for r in res.results], axis=0)
    return out.astype(np.float32)

